# revision 19
# baseline (speedup 1.0000x reference)
"""GCN layer (sparse SpMM) on 8 Trainium2 NeuronCores.

out[i] = sum_{e: rows[e]==i} vals[e] * embeds[cols[e]]   (N=10000, E=640000, D=128)

Strategy (1D row-parallel DENSE SpMM): destination rows are sharded across
the 8 cores (1250 rows each, padded to 1280). The adjacency slice is only
0.64% dense, but materializing it as a dense fp16 matrix per core
(AT[src=10112, dst=1280] ~ 26 MB) converts the per-edge gather (SWDGE
descriptor-rate-bound, ~1 us/edge) into a dense TensorE sweep at full DMA
bandwidth:

    out_c.T[feat, dst] = sum_k emb_k.T @ AT_k      (79 K-chunks of 128)

Per core on device:
  - embeds (fp16, [128, 79, 128]) and the first R_RES K-chunks of AT are
    loaded into SBUF once; the remaining 27 chunks stream per iteration
    through N_SLOTS ring-buffer slots on the ScalarE DMA ring. The slot
    count is the critical knob: the stream DMA delivers ~0.9 us/chunk vs
    PE consuming ~0.52 us/chunk, so all streamed batches must prefetch
    during the resident-chunk phase -- with only 2 slots the issue of
    batch N+2 gates on batch N's consumption and the tail of every
    iteration stalls ~5 us on DMA.
  - TensorE accumulates out.T [128 feat, 1250 dst] in 3 PSUM regions
    (512/512/226 cols; only the real 1250 dst columns are computed) over
    all 79 chunks (start/stop flags), alternating between two PSUM sets
    across repeats.
  - VectorE drains PSUM -> SBUF; SyncE DMAs the [128, 1280] fp32 out.T to
    DRAM. The host transposes back and concatenates the 8 cores.

Measured ~41.2 us/iteration (repeat-delta, see test.py) == the PE-array
streaming roofline for this decomposition (79 chunks x 1250 stream
cycles at 2.4 GHz = 41.15 us): TensorE runs at ~100% utilization.
"""

import numpy as np

N_NODES = 10000
N_EDGES = 640000
D = 128
N_CORES = 8
RPC = N_NODES // N_CORES     # 1250 destination rows per core
NPAD = 1280                  # padded dst columns (10 x 128)
KCH = 79                     # K chunks of 128 source rows (79*128 = 10112)
KPAD = KCH * 128
R_RES = 52                   # AT chunks resident in SBUF (loaded once)
B_ST = 5                     # streamed chunks per DMA batch
N_SLOTS = 4                  # stream buffer slots (prefetch depth)
S_ST = KCH - R_RES           # streamed chunks per iteration
NBATCH = (S_ST + B_ST - 1) // B_ST
# PSUM column regions for the out.T accumulator: only the 1250 real dst
# columns are computed (the AT buffers stay 1280-wide for layout).
GSL = [(0, 512), (512, 512), (1024, 226)]


def _prep_dense(adj_rows, adj_cols, adj_vals):
    """Per-core dense transposed adjacency in the device layout
    [128 part = src%128, KCH, NPAD] fp16 (accumulating duplicate edges)."""
    rows = np.asarray(adj_rows)
    cols = np.asarray(adj_cols)
    vals = np.asarray(adj_vals)
    core = rows // RPC
    ats = []
    for c in range(N_CORES):
        m = core == c
        at = np.zeros((KPAD, NPAD), np.float32)
        np.add.at(at, (cols[m], rows[m] - c * RPC), vals[m])
        ats.append(
            np.ascontiguousarray(
                at.astype(np.float16).reshape(KCH, 128, NPAD).transpose(1, 0, 2)
            )
        )
    return ats


def _prep_embeds(embeds):
    emb = np.zeros((KPAD, D), np.float16)
    emb[:N_NODES] = np.asarray(embeds).astype(np.float16)
    return np.ascontiguousarray(emb.reshape(KCH, 128, D).transpose(1, 0, 2))


def _build_program(repeat=1, b_st=B_ST, r_res=R_RES, n_slots=N_SLOTS):
    import concourse.bacc as bacc
    import concourse.mybir as mybir

    s_st = KCH - r_res
    nbatch = (s_st + b_st - 1) // b_st
    nc = bacc.Bacc("TRN2", debug=False)
    at_d = nc.dram_tensor("at", [128, KCH, NPAD], mybir.dt.float16, kind="ExternalInput")
    emb_d = nc.dram_tensor("emb", [128, KCH, D], mybir.dt.float16, kind="ExternalInput")
    out_d = nc.dram_tensor("out", [128, NPAD], mybir.dt.float32, kind="ExternalOutput")

    from contextlib import ExitStack

    with ExitStack() as stack:
        ec = stack.enter_context
        emb_s = ec(nc.sbuf_tensor("emb_s", [128, KCH, D], mybir.dt.float16))
        at_res = ec(
            nc.sbuf_tensor("at_res", [128, max(r_res, 1), NPAD], mybir.dt.float16)
        )
        at_st = ec(
            nc.sbuf_tensor("at_st", [128, n_slots * b_st, NPAD], mybir.dt.float16)
        )
        out_s = ec(nc.sbuf_tensor("out_s", [128, NPAD], mybir.dt.float32))
        psets = [
            [
                ec(nc.psum_tensor(f"p{s}{g}", [128, w], mybir.dt.float32))
                for g, w in enumerate((512, 512, 256))
            ]
            for s in range(2)
        ]
        emb_sem = ec(nc.semaphore("emb_sem"))
        res_sem = ec(nc.semaphore("res_sem"))
        at_sems = [ec(nc.semaphore(f"at_sem{s}")) for s in range(n_slots)]
        pe_batch = ec(nc.semaphore("pe_batch"))
        vcopy = ec(nc.semaphore("vcopy"))
        osem = ec(nc.semaphore("osem"))
        block = ec(nc.Block())

        @block.sync
        def _(sync):
            sync.dma_start(emb_s[:, :, :], emb_d[:, :, :]).then_inc(emb_sem, 16)
            if r_res:
                sync.dma_start(
                    at_res[:, 0:r_res, :], at_d[:, 0:r_res, :]
                ).then_inc(res_sem, 16)
            for r in range(repeat):
                sync.wait_ge(vcopy, 3 * (r + 1))
                sync.dma_start(out_d[:, :], out_s[:, :]).then_inc(osem, 16)
            sync.wait_ge(osem, repeat * 16)

        @block.scalar
        def _(scalar):
            for r in range(repeat):
                for b in range(nbatch):
                    gb = r * nbatch + b
                    if gb >= n_slots:
                        # slot gb%n_slots is free once batch gb-n_slots consumed
                        scalar.wait_ge(pe_batch, gb - n_slots + 1)
                    nch = min(b_st, s_st - b * b_st)
                    s0 = r_res + b * b_st
                    sl = (gb % n_slots) * b_st
                    scalar.dma_start(
                        at_st[:, sl:sl + nch, :], at_d[:, s0:s0 + nch, :]
                    ).then_inc(at_sems[gb % n_slots], 16)
            for s in range(n_slots):
                n = (repeat * nbatch - s + n_slots - 1) // n_slots
                if n:
                    scalar.wait_ge(at_sems[s], 16 * n)

        @block.tensor
        def _(tensor):
            tensor.wait_ge(emb_sem, 16)
            if r_res:
                tensor.wait_ge(res_sem, 16)
            for r in range(repeat):
                ps = psets[r % 2]
                if r >= 2:
                    # psum set r%2 was drained after repeat r-2's copies
                    tensor.wait_ge(vcopy, 3 * (r - 1))
                for k in range(r_res):
                    for g, (o, w) in enumerate(GSL):
                        tensor.matmul(
                            ps[g][:, 0:w],
                            emb_s[:, k, :],
                            at_res[:, k, o:o + w],
                            start=(k == 0),
                            stop=False,
                        )
                for b in range(nbatch):
                    gb = r * nbatch + b
                    tensor.wait_ge(at_sems[gb % n_slots], 16 * (gb // n_slots + 1))
                    nch = min(b_st, s_st - b * b_st)
                    sl = (gb % n_slots) * b_st
                    mm = None
                    for j in range(nch):
                        k = r_res + b * b_st + j
                        for g, (o, w) in enumerate(GSL):
                            mm = tensor.matmul(
                                ps[g][:, 0:w],
                                emb_s[:, k, :],
                                at_st[:, sl + j, o:o + w],
                                start=(r_res == 0 and k == 0),
                                stop=(k == KCH - 1),
                            )
                    mm.then_inc(pe_batch, 1)

        @block.vector
        def _(vector):
            for r in range(repeat):
                vector.wait_ge(pe_batch, (r + 1) * nbatch)
                if r >= 1:
                    # prior repeat's out DMA must finish before overwrite
                    vector.wait_ge(osem, 16 * r)
                ps = psets[r % 2]
                for g, (o, w) in enumerate(GSL):
                    vector.tensor_copy(
                        out_s[:, o:o + w], ps[g][:, 0:w]
                    ).then_inc(vcopy, 1)

    nc.compile()
    return nc


_PROG_CACHE = {}


def _get_program(repeat=1):
    if repeat not in _PROG_CACHE:
        _PROG_CACHE[repeat] = _build_program(repeat)
    return _PROG_CACHE[repeat]


def _run_with_retry(run_fn, nc, in_maps):
    # The axon-tunneled device intermittently reports
    # NRT_EXEC_UNIT_UNRECOVERABLE on the first execution of a fresh process
    # (stale state from a prior session's teardown); the failed attempt
    # resets it, so a retry usually succeeds.
    import time as _time

    last_exc = None
    for attempt in range(4):
        try:
            return run_fn(nc, in_maps, core_ids=list(range(N_CORES)))
        except Exception as e:  # noqa: BLE001
            last_exc = e
            _time.sleep(5.0 * (attempt + 1))
    raise last_exc


def kernel(adj_rows, adj_cols, adj_vals, embeds, _repeat=1, _return_raw=False):
    from concourse.bass_utils import run_bass_kernel_spmd

    ats = _prep_dense(adj_rows, adj_cols, adj_vals)
    emb_r = _prep_embeds(embeds)
    nc = _get_program(_repeat)
    in_maps = [{"at": ats[c], "emb": emb_r} for c in range(N_CORES)]
    res = _run_with_retry(run_bass_kernel_spmd, nc, in_maps)
    if _return_raw:
        return res
    return np.concatenate(
        [
            res.results[c]["out"][:, :RPC].T.astype(np.float32)
            for c in range(N_CORES)
        ],
        axis=0,
    )


# revision 20
# speedup vs baseline: 1.0181x; 1.0181x over previous
"""GCN layer (sparse SpMM) on 8 Trainium2 NeuronCores.

out[i] = sum_{e: rows[e]==i} vals[e] * embeds[cols[e]]   (N=10000, E=640000, D=128)

Strategy (1D row-parallel DENSE SpMM): destination rows are sharded across
the 8 cores (1250 rows each, padded to 1280). The adjacency slice is only
0.64% dense, but materializing it as a dense fp16 matrix per core
(AT[src=10112, dst=1280] ~ 26 MB) converts the per-edge gather (SWDGE
descriptor-rate-bound, ~1 us/edge) into a dense TensorE sweep at full DMA
bandwidth:

    out_c.T[feat, dst] = sum_k emb_k.T @ AT_k      (79 K-chunks of 128)

Per core on device:
  - embeds (fp16, [128, 79, 128]) and the first R_RES K-chunks of AT are
    loaded into SBUF once; the remaining 27 chunks stream per iteration
    through N_SLOTS ring-buffer slots on the ScalarE DMA ring. The slot
    count is the critical knob: the stream DMA delivers ~0.9 us/chunk vs
    PE consuming ~0.52 us/chunk, so all streamed batches must prefetch
    during the resident-chunk phase -- with only 2 slots the issue of
    batch N+2 gates on batch N's consumption and the tail of every
    iteration stalls ~5 us on DMA.
  - TensorE accumulates out.T [128 feat, 1250 dst] in 3 PSUM regions
    (512/512/226 cols; only the real 1250 dst columns are computed) over
    all 79 chunks (start/stop flags), alternating between two PSUM sets
    across repeats.
  - VectorE drains PSUM -> SBUF; SyncE DMAs the [128, 1280] fp32 out.T to
    DRAM. The host transposes back and concatenates the 8 cores.

Measured ~40.8-41.5 us/iteration (repeat-delta, see test.py) == the
PE-array streaming roofline for this decomposition (79 chunks x 1250
stream cycles at 2.4 GHz = 41.15 us): TensorE runs at ~100% utilization.
Skipping the ~44% of (column, chunk) pairs with no edges is
architecturally impossible (matmul streams contiguous column ranges with
fixed PSUM addressing, and the empty fraction collapses to 4% at
4-column granularity), and all sub-fp16 arithmetic paths are blocked in
this toolchain (int dtypes excluded from bass matmul; DoublePixel
unimplemented; fp8 DoubleRow fails the accuracy gate).
"""

import numpy as np

N_NODES = 10000
N_EDGES = 640000
D = 128
N_CORES = 8
RPC = N_NODES // N_CORES     # 1250 destination rows per core
NPAD = 1280                  # padded dst columns (10 x 128)
KCH = 79                     # K chunks of 128 source rows (79*128 = 10112)
KPAD = KCH * 128
R_RES = 52                   # AT chunks resident in SBUF (loaded once)
B_ST = 5                     # streamed chunks per DMA batch
N_SLOTS = 4                  # stream buffer slots (prefetch depth)
S_ST = KCH - R_RES           # streamed chunks per iteration
NBATCH = (S_ST + B_ST - 1) // B_ST
# PSUM column regions for the out.T accumulator: only the 1250 real dst
# columns are computed (the AT buffers stay 1280-wide for layout).
GSL = [(0, 512), (512, 512), (1024, 226)]


def _prep_dense(adj_rows, adj_cols, adj_vals):
    """Per-core dense transposed adjacency in the device layout
    [128 part = src%128, KCH, NPAD] fp16 (accumulating duplicate edges)."""
    rows = np.asarray(adj_rows)
    cols = np.asarray(adj_cols)
    vals = np.asarray(adj_vals)
    core = rows // RPC
    ats = []
    for c in range(N_CORES):
        m = core == c
        at = np.zeros((KPAD, NPAD), np.float32)
        np.add.at(at, (cols[m], rows[m] - c * RPC), vals[m])
        ats.append(
            np.ascontiguousarray(
                at.astype(np.float16).reshape(KCH, 128, NPAD).transpose(1, 0, 2)
            )
        )
    return ats


def _prep_embeds(embeds):
    emb = np.zeros((KPAD, D), np.float16)
    emb[:N_NODES] = np.asarray(embeds).astype(np.float16)
    return np.ascontiguousarray(emb.reshape(KCH, 128, D).transpose(1, 0, 2))


def _build_program(repeat=1, b_st=B_ST, r_res=R_RES, n_slots=N_SLOTS):
    import concourse.bacc as bacc
    import concourse.mybir as mybir

    s_st = KCH - r_res
    nbatch = (s_st + b_st - 1) // b_st
    nc = bacc.Bacc("TRN2", debug=False)
    at_d = nc.dram_tensor("at", [128, KCH, NPAD], mybir.dt.float16, kind="ExternalInput")
    emb_d = nc.dram_tensor("emb", [128, KCH, D], mybir.dt.float16, kind="ExternalInput")
    out_d = nc.dram_tensor("out", [128, NPAD], mybir.dt.float32, kind="ExternalOutput")

    from contextlib import ExitStack

    with ExitStack() as stack:
        ec = stack.enter_context
        emb_s = ec(nc.sbuf_tensor("emb_s", [128, KCH, D], mybir.dt.float16))
        at_res = ec(
            nc.sbuf_tensor("at_res", [128, max(r_res, 1), NPAD], mybir.dt.float16)
        )
        at_st = ec(
            nc.sbuf_tensor("at_st", [128, n_slots * b_st, NPAD], mybir.dt.float16)
        )
        out_s = ec(nc.sbuf_tensor("out_s", [128, NPAD], mybir.dt.float32))
        psets = [
            [
                ec(nc.psum_tensor(f"p{s}{g}", [128, w], mybir.dt.float32))
                for g, w in enumerate((512, 512, 256))
            ]
            for s in range(2)
        ]
        emb_sem = ec(nc.semaphore("emb_sem"))
        res_sem = ec(nc.semaphore("res_sem"))
        at_sems = [ec(nc.semaphore(f"at_sem{s}")) for s in range(n_slots)]
        pe_batch = ec(nc.semaphore("pe_batch"))
        vcopy = ec(nc.semaphore("vcopy"))
        osem = ec(nc.semaphore("osem"))
        block = ec(nc.Block())

        @block.sync
        def _(sync):
            sync.dma_start(emb_s[:, :, :], emb_d[:, :, :]).then_inc(emb_sem, 16)
            if r_res:
                sync.dma_start(
                    at_res[:, 0:r_res, :], at_d[:, 0:r_res, :]
                ).then_inc(res_sem, 16)
            for r in range(repeat):
                sync.wait_ge(vcopy, 3 * (r + 1))
                sync.dma_start(out_d[:, :], out_s[:, :]).then_inc(osem, 16)
            sync.wait_ge(osem, repeat * 16)

        @block.scalar
        def _(scalar):
            for r in range(repeat):
                for b in range(nbatch):
                    gb = r * nbatch + b
                    if gb >= n_slots:
                        # slot gb%n_slots is free once batch gb-n_slots consumed
                        scalar.wait_ge(pe_batch, gb - n_slots + 1)
                    nch = min(b_st, s_st - b * b_st)
                    s0 = r_res + b * b_st
                    sl = (gb % n_slots) * b_st
                    scalar.dma_start(
                        at_st[:, sl:sl + nch, :], at_d[:, s0:s0 + nch, :]
                    ).then_inc(at_sems[gb % n_slots], 16)
            for s in range(n_slots):
                n = (repeat * nbatch - s + n_slots - 1) // n_slots
                if n:
                    scalar.wait_ge(at_sems[s], 16 * n)

        @block.tensor
        def _(tensor):
            tensor.wait_ge(emb_sem, 16)
            if r_res:
                tensor.wait_ge(res_sem, 16)
            for r in range(repeat):
                ps = psets[r % 2]
                if r >= 2:
                    # psum set r%2 was drained after repeat r-2's copies
                    tensor.wait_ge(vcopy, 3 * (r - 1))
                for k in range(r_res):
                    for g, (o, w) in enumerate(GSL):
                        tensor.matmul(
                            ps[g][:, 0:w],
                            emb_s[:, k, :],
                            at_res[:, k, o:o + w],
                            start=(k == 0),
                            stop=False,
                        )
                for b in range(nbatch):
                    gb = r * nbatch + b
                    tensor.wait_ge(at_sems[gb % n_slots], 16 * (gb // n_slots + 1))
                    nch = min(b_st, s_st - b * b_st)
                    sl = (gb % n_slots) * b_st
                    mm = None
                    for j in range(nch):
                        k = r_res + b * b_st + j
                        for g, (o, w) in enumerate(GSL):
                            mm = tensor.matmul(
                                ps[g][:, 0:w],
                                emb_s[:, k, :],
                                at_st[:, sl + j, o:o + w],
                                start=(r_res == 0 and k == 0),
                                stop=(k == KCH - 1),
                            )
                    mm.then_inc(pe_batch, 1)

        @block.vector
        def _(vector):
            for r in range(repeat):
                vector.wait_ge(pe_batch, (r + 1) * nbatch)
                if r >= 1:
                    # prior repeat's out DMA must finish before overwrite
                    vector.wait_ge(osem, 16 * r)
                ps = psets[r % 2]
                for g, (o, w) in enumerate(GSL):
                    vector.tensor_copy(
                        out_s[:, o:o + w], ps[g][:, 0:w]
                    ).then_inc(vcopy, 1)

    nc.compile()
    return nc


_PROG_CACHE = {}


def _get_program(repeat=1):
    if repeat not in _PROG_CACHE:
        _PROG_CACHE[repeat] = _build_program(repeat)
    return _PROG_CACHE[repeat]


def _run_with_retry(run_fn, nc, in_maps):
    # The axon-tunneled device intermittently reports
    # NRT_EXEC_UNIT_UNRECOVERABLE on the first execution of a fresh process
    # (stale state from a prior session's teardown); the failed attempt
    # resets it, so a retry usually succeeds.
    import time as _time

    last_exc = None
    for attempt in range(4):
        try:
            return run_fn(nc, in_maps, core_ids=list(range(N_CORES)))
        except Exception as e:  # noqa: BLE001
            last_exc = e
            _time.sleep(5.0 * (attempt + 1))
    raise last_exc


def kernel(adj_rows, adj_cols, adj_vals, embeds, _repeat=1, _return_raw=False):
    from concourse.bass_utils import run_bass_kernel_spmd

    ats = _prep_dense(adj_rows, adj_cols, adj_vals)
    emb_r = _prep_embeds(embeds)
    nc = _get_program(_repeat)
    in_maps = [{"at": ats[c], "emb": emb_r} for c in range(N_CORES)]
    res = _run_with_retry(run_bass_kernel_spmd, nc, in_maps)
    if _return_raw:
        return res
    return np.concatenate(
        [
            res.results[c]["out"][:, :RPC].T.astype(np.float32)
            for c in range(N_CORES)
        ],
        axis=0,
    )
